# revision 42
# baseline (speedup 1.0000x reference)
"""ODE-RNN (nn_ODERNN_53987738911257) Trainium2 Bass kernel, v5.

Data-parallel over the N=16384 sample axis across 8 NeuronCores (2048
samples per core); hidden state lives transposed in SBUF as
[128, 4, 2048] float32 for the whole 40-observation scan.

v5 vs the 3.35ms v2 baseline -- co-designed around three measured
bottlenecks (TimelineSim): the Activation engine, the serial
merge/cast chains on DVE/GpSimd, and PSUM capacity (the 4 z-seed
accumulators at full width occupy all 8 banks, serializing chunks):

1. GEMM A (z-seed, was f32r) runs fp8 DoubleRow off the fp8 shadow of
   h: PE work drops ~35us -> ~25us per step. The shadow is kept
   post-merge-correct by converting predicated copies (f32/bf16->fp8
   in CopyPredicated).
2. The z2 accumulator re-seeds via a second fp8 A pass (A') instead of
   keeping z1's PSUM alive through the a1 tanh: halves euler PSUM
   residency so the next chunk's GEMMs flow through the 4-slot ring
   ~8us earlier.
3. Loss target folded into the p2 GEMM: an extra DoubleRow pair
   (zeros-lane, fp8(X - bp2)) against [0 | -I] weights makes the p2
   PSUM hold p - (X-bp2) directly, deleting the f32 X stream and a
   DVE subtract pass per chunk.
4. Software-pipelined emission [euler c0][euler c1][obs c0][obs c1]:
   the two 1024-column chunks are fully independent, so chunk c1's
   Act-heavy a1/a2 tanhs fill the Activation idle window while chunk
   c0 runs its h-update/cast/rnn chain, and the PSUM ring (4 slots of
   2 banks) hands chunk c0's freed C-accumulators straight to chunk
   c1's z-seeds.
5. Elementwise passes at [128,1024] (psum supertiles spanning 2
   banks -- matmuls still target 512-col bank halves as HW requires),
   halving Act/DVE instruction counts vs v2. The fp8 shadow casts
   all run on DVE, keeping the Activation engine at its floor of 32
   instructions/step -- HW measurements showed Act loading dominates
   (GpSimd is ~10x slower on real HW than the cost model claims, and
   it cannot read PSUM, so it gets no elementwise work at all).
6. Loss branch (relu -> p2 -> mask -> abs-reduce) dangles off the
   h-state critical chain (relu emitted after the rnn tanh; nothing
   downstream consumes it but the loss accumulator).
"""
import sys
sys.path.insert(0, "/opt/trn_rl_repo")

import numpy as np
import ml_dtypes

import concourse.bass as bass
import concourse.tile as tile
from concourse import bacc, mybir

F32 = mybir.dt.float32
BF16 = mybir.dt.bfloat16
FP8 = mybir.dt.float8e4
U8 = mybir.dt.uint8
U16 = mybir.dt.uint16
AF = mybir.ActivationFunctionType
ALU = mybir.AluOpType
DR = mybir.MatmulPerfMode.DoubleRow
E4M3 = ml_dtypes.float8_e4m3

P = 128
HT = 4          # hidden 512 = 4 partition tiles
CW = 1024       # elementwise column chunk width (2 PSUM banks)
MW = 512        # matmul column width (1 PSUM bank)
NCH = 2         # 2048 / 1024
NCOLS = 2048
N_CORES = 8
NSTEPS = 40
N_SAMPLES = 16384
DT = 0.05

IB_BO1, IB_BO1C, IB_BRNN, IB_BP1, IB_BP2 = 0, 4, 8, 12, 16
NB = 17


def _build_kernel(nsteps=NSTEPS, n_cores=N_CORES, reps=1):
    nc = bacc.Bacc("TRN2", target_bir_lowering=False, debug=False,
                   enable_asserts=False, num_devices=n_cores)
    x8_d = nc.dram_tensor("x8", [nsteps, P, NCOLS], FP8, kind="ExternalInput")
    xb8_d = nc.dram_tensor("xb8", [nsteps, P, NCOLS], FP8,
                           kind="ExternalInput")
    mt_d = nc.dram_tensor("mt", [nsteps, P, NCOLS], F32, kind="ExternalInput")
    mo_d = nc.dram_tensor("mo", [nsteps, P, NCOLS], U16, kind="ExternalInput")
    wa8_d = nc.dram_tensor("wa8", [2 * HT, P, 2 * P], FP8, kind="ExternalInput")
    wf8_d = nc.dram_tensor("wf8", [2 * HT, P, 2 * P], FP8, kind="ExternalInput")
    wc8_d = nc.dram_tensor("wc8", [5 * HT, P, 2 * P], FP8, kind="ExternalInput")
    wr8_d = nc.dram_tensor("wr8", [3 * HT, P, 2 * P], FP8, kind="ExternalInput")
    wp18_d = nc.dram_tensor("wp18", [2 * HT, P, 2 * P], FP8,
                            kind="ExternalInput")
    wp28_d = nc.dram_tensor("wp28", [3, P, 2 * P], FP8, kind="ExternalInput")
    b_d = nc.dram_tensor("bias", [P, NB], F32, kind="ExternalInput")
    loss_d = nc.dram_tensor("loss", [P, nsteps * NCH], F32,
                            kind="ExternalOutput")

    with tile.TileContext(nc) as tc:
        with (
            tc.tile_pool(name="const", bufs=1) as cpool,
            tc.tile_pool(name="stream", bufs=4) as spool,
            tc.tile_pool(name="work", bufs=3) as wpool,
            tc.tile_pool(name="psum", bufs=4, space="PSUM") as ppool,
        ):
            wa8 = cpool.tile([P, 2 * HT, 2, P], FP8, tag="wa8")
            wf8 = cpool.tile([P, 2 * HT, 2, P], FP8, tag="wf8")
            wc8 = cpool.tile([P, 5 * HT, 2, P], FP8, tag="wc8")
            wr8 = cpool.tile([P, 3 * HT, 2, P], FP8, tag="wr8")
            wp18 = cpool.tile([P, 2 * HT, 2, P], FP8, tag="wp18")
            wp28 = cpool.tile([P, 3, 2, P], FP8, tag="wp28")
            for i in range(2 * HT):
                nc.sync.dma_start(
                    wa8[:, i, :, :].rearrange("p a b -> p (a b)"), wa8_d[i])
                nc.sync.dma_start(
                    wf8[:, i, :, :].rearrange("p a b -> p (a b)"), wf8_d[i])
                nc.sync.dma_start(
                    wp18[:, i, :, :].rearrange("p a b -> p (a b)"), wp18_d[i])
            for i in range(5 * HT):
                nc.sync.dma_start(
                    wc8[:, i, :, :].rearrange("p a b -> p (a b)"), wc8_d[i])
            for i in range(3 * HT):
                nc.sync.dma_start(
                    wr8[:, i, :, :].rearrange("p a b -> p (a b)"), wr8_d[i])
            for i in range(3):
                nc.sync.dma_start(
                    wp28[:, i, :, :].rearrange("p a b -> p (a b)"), wp28_d[i])
            bia = cpool.tile([P, NB], F32, tag="bias")
            nc.sync.dma_start(bia[:], b_d[:])

            hT = cpool.tile([P, HT, NCOLS], F32, tag="hT")
            # lanes: 0 = x8 (per-obs DMA), 1..4 = fp8 cast of hT,
            # 5 = zeros, 6 = xb8 = fp8(X - bp2) (per-obs DMA; pairs
            # with the zeros lane as p2's -Identity loss-target pair)
            h8x = cpool.tile([P, 7, NCOLS], FP8, tag="h8x")
            # lanes: 0..3 = a1, 4..7 = a2, 8 = ones (bias), 9 = zeros
            act8 = cpool.tile([P, 10, NCOLS], FP8, tag="act8")
            p18 = cpool.tile([P, HT, NCOLS], FP8, tag="p18")
            loss_sb = cpool.tile([P, nsteps * NCH], F32, tag="loss")

            def bcol(i):
                return bia[:, i:i + 1]

            obs_tiles = {}

            def get_obs(k):
                if k not in obs_tiles:
                    mt = spool.tile([P, NCOLS], F32, tag="mt")
                    nc.sync.dma_start(mt[:], mt_d[k])
                    mo = spool.tile([P, NCOLS], U16, tag="mo")
                    nc.sync.dma_start(mo[:], mo_d[k])
                    obs_tiles[k] = (mt, mo)
                return obs_tiles[k]

            def emit_euler_z1(k, c):
                cs = bass.ts(c, CW)
                # z1 = (Wo1/dt)^T h8  (fp8 DR), freed right after tanh a1
                z1 = []
                for jt in range(HT):
                    ps = ppool.tile([P, CW], F32, tag="ps")
                    for h in range(2):
                        sl = bass.ts(2 * c + h, MW)
                        for g in range(2):
                            nc.tensor.matmul(
                                ps[:, bass.ts(h, MW)],
                                wa8[:, g * HT + jt, :, :],
                                h8x[:, 1 + 2 * g:3 + 2 * g, sl],
                                start=(g == 0), stop=(g == 1), perf_mode=DR,
                                skip_group_check=True)
                    z1.append(ps)
                for jt in range(HT):
                    nc.scalar.activation(act8[:, jt, cs], z1[jt][:], AF.Tanh,
                                         bias=bcol(IB_BO1 + jt), scale=DT)

            def emit_euler_z2(k, c):
                cs = bass.ts(c, CW)
                # z2 = (Wo1/dt)^T h8 + (Wo1@Wo2)^T a1  (A' recompute + B)
                z2 = []
                for jt in range(HT):
                    ps = ppool.tile([P, CW], F32, tag="ps")
                    for h in range(2):
                        sl = bass.ts(2 * c + h, MW)
                        for g in range(2):
                            nc.tensor.matmul(
                                ps[:, bass.ts(h, MW)],
                                wa8[:, g * HT + jt, :, :],
                                h8x[:, 1 + 2 * g:3 + 2 * g, sl],
                                start=(g == 0), stop=False, perf_mode=DR,
                                skip_group_check=True)
                        for g in range(2):
                            nc.tensor.matmul(
                                ps[:, bass.ts(h, MW)],
                                wf8[:, g * HT + jt, :, :],
                                act8[:, 2 * g:2 * g + 2, sl],
                                start=False, stop=(g == 1), perf_mode=DR,
                                skip_group_check=True)
                    z2.append(ps)
                for jt in range(HT):
                    nc.scalar.activation(act8[:, HT + jt, cs], z2[jt][:],
                                         AF.Tanh, bias=bcol(IB_BO1C + jt),
                                         scale=DT)

            def emit_euler_c(k, c, jts=range(HT)):
                cs = bass.ts(c, CW)
                # GEMM C: ps_h = Wo2^T (a1 + a2) + 2*bo2 (bias lane pair)
                for jt in jts:
                    ps = ppool.tile([P, CW], F32, tag="ps")
                    for h in range(2):
                        sl = bass.ts(2 * c + h, MW)
                        for l in range(5):
                            nc.tensor.matmul(
                                ps[:, bass.ts(h, MW)],
                                wc8[:, l * HT + jt, :, :],
                                act8[:, 2 * l:2 * l + 2, sl],
                                start=(l == 0), stop=(l == 4), perf_mode=DR,
                                skip_group_check=True)
                    # h += dt * ps_h, then refresh the fp8 shadow; jt-major
                    # so C(jt0)'s PSUM frees first and the next chunk's z1
                    # GEMMs flow through the ring early
                    nc.vector.scalar_tensor_tensor(
                        out=hT[:, jt, cs], in0=ps[:], scalar=DT,
                        in1=hT[:, jt, cs], op0=ALU.mult, op1=ALU.add)
                    nc.vector.tensor_scalar(
                        h8x[:, 1 + jt, cs], hT[:, jt, cs], 0.0, None,
                        ALU.add)

            def emit_obsgemm(k, c):
                mt, mo = get_obs(k)
                # rnn + p1 GEMMs interleaved per jt (ring: rnn reuses C slots
                # freed by the stt halves, p1 the next); rnn tanh per jt so
                # the merge chain starts after the first rnn tile
                rps, pps = [], []
                for jt in range(HT):
                    ps = ppool.tile([P, CW], F32, tag="ps")
                    for h in range(2):
                        sl = bass.ts(2 * c + h, MW)
                        for g in range(3):
                            nc.tensor.matmul(
                                ps[:, bass.ts(h, MW)],
                                wr8[:, g * HT + jt, :, :],
                                h8x[:, 2 * g:2 * g + 2, sl],
                                start=(g == 0), stop=(g == 2), perf_mode=DR,
                                skip_group_check=True)
                    rps.append(ps)
                    ps = ppool.tile([P, CW], F32, tag="ps")
                    for h in range(2):
                        sl = bass.ts(2 * c + h, MW)
                        for g in range(2):
                            nc.tensor.matmul(
                                ps[:, bass.ts(h, MW)],
                                wp18[:, g * HT + jt, :, :],
                                h8x[:, 1 + 2 * g:3 + 2 * g, sl],
                                start=(g == 0), stop=(g == 1), perf_mode=DR,
                                skip_group_check=True)
                    pps.append(ps)
                hns = wpool.tile([P, HT, CW], F32, tag="hns")
                for jt in range(HT):
                    nc.scalar.activation(hns[:, jt, :], rps[jt][:], AF.Tanh,
                                         bias=bcol(IB_BRNN + jt))
                # fp8-shadow merges (they gate next step's GEMM A, which is
                # a full pipelined chunk away -- full width is cheapest)
                cs = bass.ts(c, CW)
                for jt in range(HT):
                    nc.vector.copy_predicated(
                        h8x[:, 1 + jt, cs], mo[:, cs], hns[:, jt, :])
                return hns, pps

            def emit_lossbranch(k, c, pps):
                mt, mo = get_obs(k)
                cs = bass.ts(c, CW)
                for jt in range(HT):
                    nc.scalar.activation(p18[:, jt, cs], pps[jt][:], AF.Relu,
                                         bias=bcol(IB_BP1 + jt))
                # pred - (X - bp2): the 3rd DR pair is (zeros, xb8) against
                # [0 | -I] weights, so ps_w holds dm directly
                ps_w = ppool.tile([P, CW], F32, tag="ps")
                for h in range(2):
                    sl = bass.ts(2 * c + h, MW)
                    for g in range(2):
                        nc.tensor.matmul(
                            ps_w[:, bass.ts(h, MW)], wp28[:, g, :, :],
                            p18[:, 2 * g:2 * g + 2, sl],
                            start=(g == 0), stop=False, perf_mode=DR,
                            skip_group_check=True)
                    nc.tensor.matmul(
                        ps_w[:, bass.ts(h, MW)], wp28[:, 2, :, :],
                        h8x[:, 5:7, sl],
                        start=False, stop=True, perf_mode=DR,
                        skip_group_check=True)
                return ps_w

            def emit_mergehT(k, c, hns):
                # f32 hT merges: only gate the next step's h-update stt,
                # ~10us of slack away, so they trail the shadow merges
                _, mo = obs_tiles[k]
                cs = bass.ts(c, CW)
                for jt in range(HT):
                    nc.vector.copy_predicated(
                        hT[:, jt, cs], mo[:, cs], hns[:, jt, :])

            def emit_loss_mask(k, c, ps_w):
                # mask-mult stays in place: it frees the p2 PSUM ring slot
                # that gates downstream GEMM allocations
                mt, _ = obs_tiles[k]
                cs = bass.ts(c, CW)
                sc = wpool.tile([P, CW], F32, tag="sc")
                nc.vector.tensor_tensor(sc[:], ps_w[:], mt[:, cs], ALU.mult)
                return sc

            def emit_loss_reduce(k, c, sc):
                # the abs-reduce is SBUF-only and feeds nothing but the loss
                # output -- deferred past both chunks' merges so it never
                # sits between them on the in-order DVE queue
                nc.vector.tensor_reduce(
                    loss_sb[:, k * NCH + c: k * NCH + c + 1], sc[:],
                    mybir.AxisListType.X, ALU.add, apply_absolute_value=True)

            nc.vector.memset(hT[:], 0.0)
            nc.vector.memset(h8x[:, 1:7, :].bitcast(U8), 0)
            nc.vector.memset(act8[:, 8, :], 1.0)
            nc.vector.memset(act8[:, 9, :].bitcast(U8), 0)
            for rep in range(reps):
                for k in range(nsteps):
                    get_obs(k)
                    nc.sync.dma_start(h8x[:, 0, :], x8_d[k])
                    nc.sync.dma_start(h8x[:, 6, :], xb8_d[k])
                    # Emission (= per-engine program order and PSUM ring
                    # order) is software-pipelined across the two
                    # independent column chunks: chunk c1's z-seed GEMMs
                    # dispatch before chunk c0's C GEMMs so the a1(c1)
                    # tanhs follow a2(c0) immediately; chunk c0's rnn/p1
                    # PSUM tiles are allocated before chunk c1's C tiles so
                    # the ring hands them z2(c1) slots (freed by the a2
                    # tanhs) instead of gating them on chunk c1's h-update.
                    emit_euler_z1(k, 0)
                    emit_euler_z2(k, 0)
                    emit_euler_z1(k, 1)
                    emit_euler_c(k, 0, jts=(0, 1))
                    emit_euler_z2(k, 1)
                    emit_euler_c(k, 0, jts=(2, 3))
                    emit_euler_c(k, 1)
                    scs = []
                    for c in range(NCH):
                        hns, pps = emit_obsgemm(k, c)
                        ps_w = emit_lossbranch(k, c, pps)
                        # mask before the hT merges: it frees the p2 PSUM
                        # ring slot that gates the next step's 4th z-seed,
                        # while the hT merges have ~12us of consumer slack
                        scs.append(emit_loss_mask(k, c, ps_w))
                        emit_mergehT(k, c, hns)
                    for c in range(NCH):
                        emit_loss_reduce(k, c, scs[c])
                    del obs_tiles[k]
                if rep + 1 < reps:
                    nc.vector.memset(hT[:], 0.0)
                    nc.vector.memset(h8x[:, 1:7, :].bitcast(U8), 0)

            nc.sync.dma_start(loss_d[:], loss_sb[:])
    nc.compile()
    return nc


def _wtiles(W):
    """[out, in] torch-layout weight -> [ko, jo, P, P] PE tiles of W.T."""
    WT = np.ascontiguousarray(np.asarray(W, np.float32).T)
    ko, jo = WT.shape[0] // P, WT.shape[1] // P
    return np.ascontiguousarray(WT.reshape(ko, P, jo, P).transpose(0, 2, 1, 3))


def _pair8(arr):
    """[ko, jo, P, P] (ko even) -> fp8 [ko//2 * jo, P, 2P] DoubleRow pairs."""
    ko, jo = arr.shape[0], arr.shape[1]
    out = arr.reshape(ko // 2, 2, jo, P, P).transpose(0, 2, 3, 1, 4)
    out = out.reshape(ko // 2 * jo, P, 2 * P)
    return np.ascontiguousarray(out).astype(E4M3)


def _prep_inputs(X, M, batch_idx, W_ih, b_ih, W_hh, b_hh,
                 Wo1, bo1, Wo2, bo2, Wp1, bp1, Wp2, bp2):
    X = np.asarray(X, np.float32)
    M = np.asarray(M, np.float32)
    batch_idx = np.asarray(batch_idx)
    Wo1 = np.asarray(Wo1, np.float32)
    Wo2 = np.asarray(Wo2, np.float32)
    bo1 = np.asarray(bo1, np.float32)
    bo2 = np.asarray(bo2, np.float32)
    K = X.shape[0]
    npc = N_SAMPLES // N_CORES

    wa8 = _pair8(_wtiles(Wo1 / DT))
    wf8 = _pair8(_wtiles(Wo1 @ Wo2))
    # C weights: lanes a1 (Wo2 pairs), a2 (Wo2 pairs), bias pair:
    # lane8 weight row p==0 = 2*bo2 (rhs lane8 is all-ones), lane9 zero.
    p2 = _pair8(_wtiles(Wo2))
    bias8 = np.zeros((HT, P, 2, P), np.float32)
    for jt in range(HT):
        bias8[jt, 0, 0, :] = 2.0 * bo2[jt * P:(jt + 1) * P]
    bias8 = bias8.reshape(HT, P, 2 * P).astype(E4M3)
    wc8 = np.concatenate([p2, p2, bias8], axis=0)

    whh = _wtiles(W_hh)
    wih = _wtiles(W_ih)  # [1, HT, P, P]
    # rnn lane pairs: (wih, whh0), (whh1, whh2), (whh3, 0)
    rnn = np.zeros((3, 2, HT, P, P), np.float32)
    rnn[0, 0] = wih[0]
    rnn[0, 1] = whh[0]
    rnn[1, 0] = whh[1]
    rnn[1, 1] = whh[2]
    rnn[2, 0] = whh[3]
    wr8 = np.ascontiguousarray(
        rnn.transpose(0, 2, 3, 1, 4).reshape(3 * HT, P, 2 * P)).astype(E4M3)

    wp18 = _pair8(_wtiles(Wp1))
    # 3rd pair: (zeros-lane, xb8-lane) x [0 | -I] subtracts the loss target
    wneg = np.zeros((1, 2, P, P), np.float32)
    wneg[0, 1] = -np.eye(P, dtype=np.float32)
    wneg = wneg.transpose(0, 2, 1, 3).reshape(1, P, 2 * P).astype(E4M3)
    wp28 = np.concatenate([_pair8(_wtiles(Wp2)), wneg], axis=0)

    bias = np.zeros((P, NB), np.float32)
    bo1c = bo1 + DT * (Wo1 @ bo2)
    bias[:, IB_BO1:IB_BO1 + 4] = bo1.reshape(4, P).T
    bias[:, IB_BO1C:IB_BO1C + 4] = bo1c.reshape(4, P).T
    brnn = np.asarray(b_ih, np.float32) + np.asarray(b_hh, np.float32)
    bias[:, IB_BRNN:IB_BRNN + 4] = brnn.reshape(4, P).T
    bias[:, IB_BP1:IB_BP1 + 4] = np.asarray(bp1, np.float32).reshape(4, P).T
    bias[:, IB_BP2] = np.asarray(bp2, np.float32)

    kk = np.arange(K)[:, None]
    Xs = np.zeros((K, N_SAMPLES, X.shape[2]), np.float32)
    Xs[kk, batch_idx] = X
    Ms = np.zeros((K, N_SAMPLES, X.shape[2]), np.float32)
    Ms[kk, batch_idx] = M
    obs = np.zeros((K, N_SAMPLES), np.float32)
    obs[kk, batch_idx] = 1.0

    bp2f = np.asarray(bp2, np.float32)
    in_maps = []
    for c in range(N_CORES):
        slc = slice(c * npc, (c + 1) * npc)
        xt = np.ascontiguousarray(Xs[:, slc].transpose(0, 2, 1))
        x8 = xt.astype(E4M3)
        # fold bp2 into the loss target: dm = p2_psum - (X - bp2)
        xb8 = (xt - bp2f[None, :, None]).astype(E4M3)
        mtc = np.ascontiguousarray(Ms[:, slc].transpose(0, 2, 1))
        moc = np.ascontiguousarray(np.broadcast_to(
            obs[:, None, slc], (K, P, npc))).astype(np.uint16)
        in_maps.append({
            "x8": x8, "xb8": xb8, "mt": mtc, "mo": moc,
            "wa8": wa8, "wf8": wf8, "wc8": wc8, "wr8": wr8,
            "wp18": wp18, "wp28": wp28, "bias": bias,
        })
    tot_m = float(np.asarray(M, np.float64).sum())
    return in_maps, tot_m


class _Runner:
    """Compile once per process; re-usable across kernel() calls."""

    def __init__(self, nc, n_cores):
        import jax
        from jax.sharding import Mesh, PartitionSpec, NamedSharding
        from jax.experimental.shard_map import shard_map
        from concourse.bass2jax import (
            _bass_exec_p, install_neuronx_cc_hook, partition_id_tensor)
        install_neuronx_cc_hook()
        self.jax = jax
        self.n_cores = n_cores
        partition_name = (
            nc.partition_id_tensor.name if nc.partition_id_tensor else None)
        in_names, out_names, out_avals, zero_outs = [], [], [], []
        for alloc in nc.m.functions[0].allocations:
            if not isinstance(alloc, mybir.MemoryLocationSet):
                continue
            name = alloc.memorylocations[0].name
            if alloc.kind == "ExternalInput":
                if name != partition_name:
                    in_names.append(name)
            elif alloc.kind == "ExternalOutput":
                shape = tuple(alloc.tensor_shape)
                dtype = mybir.dt.np(alloc.dtype)
                out_names.append(name)
                out_avals.append(jax.core.ShapedArray(shape, dtype))
                zero_outs.append(np.zeros(shape, dtype))
        self.in_names = in_names
        self.out_names = out_names
        self.out_avals = out_avals
        self.zero_outs = zero_outs
        n_params = len(in_names)
        n_outs = len(out_avals)
        all_in_names = in_names + out_names
        if partition_name is not None:
            all_in_names.append(partition_name)

        def _body(*args):
            operands = list(args)
            if partition_name is not None:
                operands.append(partition_id_tensor())
            outs = _bass_exec_p.bind(
                *operands,
                out_avals=tuple(out_avals),
                in_names=tuple(all_in_names),
                out_names=tuple(out_names),
                lowering_input_output_aliases=(),
                sim_require_finite=True,
                sim_require_nnan=True,
                nc=nc,
            )
            return tuple(outs)

        devices = jax.devices()[:n_cores]
        assert len(devices) == n_cores, \
            f"need {n_cores} neuron cores, found {len(jax.devices())}"
        self.mesh = Mesh(np.asarray(devices), ("core",))
        in_specs = (PartitionSpec("core"),) * (n_params + n_outs)
        out_specs = (PartitionSpec("core"),) * n_outs
        self.fn = jax.jit(
            shard_map(_body, mesh=self.mesh, in_specs=in_specs,
                      out_specs=out_specs, check_rep=False),
            keep_unused=True)
        self.sharding = NamedSharding(self.mesh, PartitionSpec("core"))

    def run(self, in_maps):
        jax = self.jax
        devices = list(self.mesh.devices.flat)
        dev_inputs = []
        for n in self.in_names:
            shards = [jax.device_put(np.asarray(in_maps[c][n]), devices[c])
                      for c in range(self.n_cores)]
            s0 = shards[0].shape
            dev_inputs.append(jax.make_array_from_single_device_arrays(
                (self.n_cores * s0[0], *s0[1:]), self.sharding, shards))
        for z in self.zero_outs:
            shards = [jax.device_put(np.zeros(z.shape, z.dtype), devices[c])
                      for c in range(self.n_cores)]
            dev_inputs.append(jax.make_array_from_single_device_arrays(
                (self.n_cores * z.shape[0], *z.shape[1:]),
                self.sharding, shards))
        outs = self.fn(*dev_inputs)
        jax.block_until_ready(outs)
        return [
            {name: np.asarray(outs[i]).reshape(
                self.n_cores, *self.out_avals[i].shape)[c]
             for i, name in enumerate(self.out_names)}
            for c in range(self.n_cores)
        ]


_runner = None


def _get_runner():
    global _runner
    if _runner is None:
        nc = _build_kernel()
        _runner = _Runner(nc, N_CORES)
    return _runner


def kernel(X, M, batch_idx, W_ih, b_ih, W_hh, b_hh,
           Wo1, bo1, Wo2, bo2, Wp1, bp1, Wp2, bp2):
    in_maps, tot_m = _prep_inputs(
        X, M, batch_idx, W_ih, b_ih, W_hh, b_hh,
        Wo1, bo1, Wo2, bo2, Wp1, bp1, Wp2, bp2)
    results = _get_runner().run(in_maps)
    loss = sum(float(r["loss"].astype(np.float64).sum()) for r in results)
    return np.array([loss, loss / tot_m], np.float32)
